# revision 5
# baseline (speedup 1.0000x reference)
"""CenterLoss kernel for Trainium2, data-parallel across 8 NeuronCores.

Math: reference computes the full [B, C] squared-distance matrix, masks it
with one_hot(labels), clamps to [1e-12, 1e12] and sums.  The mask keeps only
distmat[i, labels[i]]; every other entry becomes clip(0) = 1e-12.  So

    loss = ( sum_i clip(||x_i - c_{l_i}||^2) + B*(C-1)*1e-12 ) / B

Per core (B/8 = 2048 rows): gather c_{l_i} rows from DRAM via indirect DMA,
diff = x - g (VectorE), Square+row-accumulate (ScalarE ACT) -> per-row
distance, clamp, reduce -> [128, 1] partials.  Host sums partials.
"""

import sys

import numpy as np

try:
    import concourse.bass  # noqa: F401
except ImportError:
    sys.path.insert(0, "/opt/trn_rl_repo")

import concourse.bass as bass
import concourse.mybir as mybir
from concourse.bacc import Bacc
from concourse.bass_utils import run_bass_kernel_spmd
from concourse.tile import TileContext

B, C, D = 16384, 1000, 512
N_CORES = 8
B_SHARD = B // N_CORES  # 2048
P = 128
N_CHUNKS = B_SHARD // P  # 16
CLAMP_MIN = 1e-12
CLAMP_MAX = 1e12

_NC_CACHE = {}


def build_nc():
    nc = Bacc()
    x_d = nc.declare_dram_parameter("x", [B_SHARD, D], mybir.dt.float32, isOutput=False)
    lbl_d = nc.declare_dram_parameter(
        "labels", [P, N_CHUNKS], mybir.dt.int32, isOutput=False
    )
    cen_d = nc.declare_dram_parameter(
        "centers", [C, D], mybir.dt.float32, isOutput=False
    )
    out_d = nc.declare_dram_parameter("out", [P, 1], mybir.dt.float32, isOutput=True)

    with TileContext(nc) as tc:
        with (
            tc.tile_pool(name="persist", bufs=1) as persist,
            tc.tile_pool(name="io", bufs=4) as io,
            tc.tile_pool(name="work", bufs=4) as work,
        ):
            lbl_tile = persist.tile([P, N_CHUNKS], mybir.dt.int32)
            nc.sync.dma_start(out=lbl_tile[:], in_=lbl_d[:])
            acc = persist.tile([P, N_CHUNKS], mybir.dt.float32)

            for g in range(N_CHUNKS):
                x_tile = io.tile([P, D], mybir.dt.float32, tag="x")
                nc.sync.dma_start(out=x_tile[:], in_=x_d[g * P : (g + 1) * P, :])
                g_tile = io.tile([P, D], mybir.dt.float32, tag="g")
                nc.gpsimd.indirect_dma_start(
                    out=g_tile[:],
                    out_offset=None,
                    in_=cen_d[:],
                    in_offset=bass.IndirectOffsetOnAxis(
                        ap=lbl_tile[:, g : g + 1], axis=0
                    ),
                )
                diff = work.tile([P, D], mybir.dt.float32, tag="diff")
                nc.vector.tensor_tensor(
                    out=diff[:],
                    in0=x_tile[:],
                    in1=g_tile[:],
                    op=mybir.AluOpType.subtract,
                )
                sq = work.tile([P, D], mybir.dt.float32, tag="sq")
                nc.scalar.activation(
                    out=sq[:],
                    in_=diff[:],
                    func=mybir.ActivationFunctionType.Square,
                    accum_out=acc[:, g : g + 1],
                )

            # clamp per-row distances exactly like the reference clip
            nc.vector.tensor_scalar(
                out=acc[:],
                in0=acc[:],
                scalar1=CLAMP_MIN,
                scalar2=CLAMP_MAX,
                op0=mybir.AluOpType.max,
                op1=mybir.AluOpType.min,
            )
            out_tile = persist.tile([P, 1], mybir.dt.float32)
            nc.vector.tensor_reduce(
                out=out_tile[:],
                in_=acc[:],
                axis=mybir.AxisListType.X,
                op=mybir.AluOpType.add,
            )
            nc.sync.dma_start(out=out_d[:], in_=out_tile[:])
    nc.finalize()
    return nc


def _get_nc():
    if "nc" not in _NC_CACHE:
        _NC_CACHE["nc"] = build_nc()
    return _NC_CACHE["nc"]


def kernel(x, labels, centers, _trace=False):
    x = np.asarray(x, dtype=np.float32)
    centers = np.asarray(centers, dtype=np.float32)
    labels_i32 = np.asarray(labels, dtype=np.int32)

    in_maps = []
    for i in range(N_CORES):
        xs = np.ascontiguousarray(x[i * B_SHARD : (i + 1) * B_SHARD])
        ls = labels_i32[i * B_SHARD : (i + 1) * B_SHARD]
        # [P, N_CHUNKS] layout: column g holds labels of rows g*128..(g+1)*128
        ls = np.ascontiguousarray(ls.reshape(N_CHUNKS, P).T)
        in_maps.append({"x": xs, "labels": ls, "centers": centers})

    nc = _get_nc()
    res = run_bass_kernel_spmd(nc, in_maps, list(range(N_CORES)), trace=_trace)
    partials = np.stack([r["out"] for r in res.results])  # [8, 128, 1]
    total = np.sum(partials.astype(np.float64))
    total += B * (C - 1) * CLAMP_MIN
    loss = np.float32(total / B)
    if _trace:
        return np.asarray(loss), res
    return np.asarray(loss)
